# revision 31
# baseline (speedup 1.0000x reference)
"""Trainium2 kernel for nn_EnhancedHybridModel (hybrid MLP + 8-qubit circuit).

Reformulation (exact, up to f32 rounding):
  * BatchNorms are inference-mode -> folded into the adjacent Linear.
  * The quantum circuit after RY-encoding uses shared weights, so it is one
    fixed complex matrix M (256x256).  The encoded state is a REAL product
    vector s[b] = kron_i [cos(pre_i/2), -sin(pre_i/2)].
  * q_out = |M s|^2 @ Z  ->  y = [Re M; Im M] @ s  (512x256 matmul),
    then q_out @ W4eff.T folds with the Z-projection into M4 (512x32):
    h4 = relu(y^2 @ M4 + b4eff).

Data parallel over 8 NeuronCores: batch 65536 -> 8192 rows/core.
v2 pipeline: the pre-angles are computed BATCH-major (stationary = h2
sub-blocks) so cos/sin come from two scalar-engine Sin LUT ops and no
feature-major->batch-major transposes are needed.  The product-state build
stays in batch-major (strided broadcast krons on DVE/Pool), is transposed
back by 8 PE-transposes into one fp16 PSUM bank, and evacuated by a single
DVE copy.  All matmuls fp16.
"""

import numpy as np

import concourse.bass as bass
import concourse.mybir as mybir
import concourse.tile as tile
from concourse import bacc
from concourse.hw_specs import get_activation_tables
from concourse.masks import make_identity
from concourse.bass_utils import run_bass_kernel_spmd

F32 = mybir.dt.float32
F16 = mybir.dt.float16
AF = mybir.ActivationFunctionType
ALU = mybir.AluOpType

N_CORES = 8
BATCH = 65536
B_CORE = BATCH // N_CORES  # 8192
COLS = 512  # batch columns per tile (one PSUM bank of f32)
NTILES = B_CORE // COLS  # 16

N_QUBITS = 8
N_LAYERS = 3
DIM = 256
EPS = 1e-5

# ---------------------------------------------------------------- host math

_idx = np.arange(DIM)
_CNOT_PERMS = []
for _i in range(N_QUBITS):
    for _j in range(_i + 1, N_QUBITS):
        _c = (_idx >> (N_QUBITS - 1 - _i)) & 1
        _CNOT_PERMS.append(np.where(_c == 1, _idx ^ (1 << (N_QUBITS - 1 - _j)), _idx))
_Z_SIGNS = np.stack(
    [1.0 - 2.0 * ((_idx >> (N_QUBITS - 1 - i)) & 1) for i in range(N_QUBITS)], axis=1
).astype(np.float64)


def _rx(t):
    c, s = np.cos(t / 2), -1j * np.sin(t / 2)
    return np.array([[c, s], [s, c]], np.complex128)


def _ry(t):
    c, s = np.cos(t / 2), np.sin(t / 2)
    return np.array([[c, -s], [s, c]], np.complex128)


def _rz(t):
    e = np.exp(-0.5j * t)
    return np.array([[e, 0], [0, np.conj(e)]], np.complex128)


def _apply_gate(M, G, w):
    # reference einsum('st,bpsq->bptq', U, state): state'[t] = sum_s U[s,t] state[s]
    left = 2**w
    Mr = M.reshape(left, 2, -1, DIM)
    return np.einsum("st,psqj->ptqj", G, Mr).reshape(DIM, DIM)


def _build_circuit_matrix(q_weights):
    qw = np.asarray(q_weights, np.float64)
    M = np.eye(DIM, dtype=np.complex128)
    for l in range(N_LAYERS):
        for i in range(N_QUBITS):
            M = _apply_gate(M, _rx(qw[l, i, 0]), i)
            M = _apply_gate(M, _ry(qw[l, i, 1]), i)
            M = _apply_gate(M, _rz(qw[l, i, 2]), i)
        for perm in _CNOT_PERMS:
            M = M[perm, :]
    return M


def _fold_bn(W, b, g, bt, m, v):
    sc = np.asarray(g, np.float64) / np.sqrt(np.asarray(v, np.float64) + EPS)
    Weff = sc[:, None] * np.asarray(W, np.float64)
    beff = (np.asarray(b, np.float64) - np.asarray(m, np.float64)) * sc + np.asarray(
        bt, np.float64
    )
    return Weff, beff


def _prep_consts(inputs):
    f = {k: np.asarray(v, np.float64) for k, v in inputs.items() if k != "x"}
    W1e, b1e = _fold_bn(f["W1"], f["b1"], f["g1"], f["bt1"], f["m1"], f["v1"])
    W2e, b2e = _fold_bn(f["W2"], f["b2"], f["g2"], f["bt2"], f["m2"], f["v2"])
    W4e, b4e = _fold_bn(f["W4"], f["b4"], f["g4"], f["bt4"], f["m4"], f["v4"])
    M = _build_circuit_matrix(f["q_weights"])
    C = np.concatenate([M.real, M.imag], axis=0)  # (512, 256)
    Zst = np.concatenate([_Z_SIGNS, _Z_SIGNS], axis=0)  # (512, 8)
    M4 = Zst @ W4e.T  # (512, 32)

    bf = np.float16
    # WPACK fp16 [128, 1377]: ct | w2 | w1 | w3 | w5 | w6 | m4  (row-padded)
    wpk = np.zeros((128, 1377), bf)
    CT = np.ascontiguousarray(C.T).astype(bf)  # (256,512)
    wpk[:, 0:512] = CT[0:128]
    wpk[:, 512:1024] = CT[128:256]
    wpk[0:128, 1024:1088] = np.ascontiguousarray(W2e.T).astype(bf)
    wpk[0:16, 1088:1216] = np.ascontiguousarray(W1e.T).astype(bf)
    w3t = np.concatenate([f["W3"].T, f["W3"].T], axis=1)  # (64,16)
    wpk[0:64, 1216:1232] = np.ascontiguousarray(w3t).astype(bf)
    wpk[0:32, 1232:1248] = np.ascontiguousarray(f["W5"].T).astype(bf)
    wpk[0:16, 1248:1249] = np.ascontiguousarray(f["W6"].T).astype(bf)
    M4b = M4.astype(bf)  # (512,32)
    for c in range(4):
        wpk[:, 1249 + 32 * c : 1249 + 32 * (c + 1)] = M4b[128 * c : 128 * (c + 1)]
    # BIASES f32 [128, 16]: b1 b2 b3 b4 b5 b6
    bs = np.zeros((128, 16), np.float32)
    bs[0:128, 0] = b1e
    bs[0:64, 1] = b2e
    bs[0:16, 2] = np.concatenate([f["b3"], f["b3"]])
    bs[0:32, 3] = b4e
    bs[0:16, 4] = f["b5"]
    bs[0:1, 5] = f["b6"]
    # cos-as-sin: rows 0-7 sin(0.5p + pi/2) = cos(p/2); rows 8-15 sin(-0.5p)
    bs[0:8, 6] = np.pi / 2
    bs[0:8, 7] = 0.5
    bs[8:16, 7] = -0.5
    return {"WPACK": wpk, "BIASES": bs}


# ------------------------------------------------------------- bass program


def _ap(t, offset, dims):
    """Custom free-dim access pattern on a tile: keep its partition dim."""
    a = t[:]
    return bass.AP(a.tensor, a.offset + offset, [list(a.ap[0])] + [list(d) for d in dims])


HALF_PI = float(np.pi / 2)


def _build_nc():
    nc = bacc.Bacc("TRN2", target_bir_lowering=False, debug=False)

    xt = nc.dram_tensor("xt", [16, B_CORE], F16, kind="ExternalInput")
    wpk_d = nc.dram_tensor("WPACK", [128, 1377], F16, kind="ExternalInput")
    bs_d = nc.dram_tensor("BIASES", [128, 16], F32, kind="ExternalInput")
    out_d = nc.dram_tensor("out", [1, B_CORE], F32, kind="ExternalOutput")

    with tile.TileContext(nc) as tc:
        with (
            tc.tile_pool(name="const", bufs=1) as cp,
            tc.tile_pool(name="work", bufs=4) as wp,
            tc.tile_pool(name="pmlp", bufs=2, space="PSUM") as pmlp,
            tc.tile_pool(name="py", bufs=4, space="PSUM") as py,
            tc.tile_pool(name="pmlb", bufs=2, space="PSUM") as pmlb,
        ):
            # Pre-load one activation table set covering every LUT we use, so
            # the fixpoint table-load pass doesn't thrash between sets
            # (Tanh->exp_and_others vs Sin->trig_and_small at 1283ns/reload).
            _need = {AF.Tanh, AF.Sin, AF.Square, AF.Identity}
            _tabs = list(get_activation_tables(nc.m.arch).items())
            _set_id = next(i for i, (_n, _s) in enumerate(_tabs) if _need <= _s)
            nc.scalar.add_instruction(
                mybir.InstLoadActFuncSet(
                    name=nc.get_next_instruction_name(), ins=[], outs=[],
                    act_func_set_id=_set_id,
                )
            )

            wpk = cp.tile([128, 1377], F16)
            nc.scalar.dma_start(wpk[:], wpk_d[:])
            bs = cp.tile([128, 16], F32)
            nc.sync.dma_start(bs[:], bs_d[:])
            ct = wpk[:, 0:1024]
            w2 = wpk[:, 1024:1088]
            w1 = wpk[0:16, 1088:1216]
            w3 = wpk[0:64, 1216:1232]
            w5 = wpk[0:32, 1232:1248]
            w6 = wpk[0:16, 1248:1249]
            m4 = wpk[:, 1249:1377]
            bias = {
                "b1": bs[0:128, 0:1], "b2": bs[0:64, 1:2], "b3": bs[0:16, 2:3],
                "b4": bs[0:32, 3:4], "b5": bs[0:16, 4:5],
                "sinb": bs[0:16, 6:7], "sins": bs[0:16, 7:8],
            }
            xg = []
            for g in range(4):
                xg.append(cp.tile([16, 4 * COLS], F16, name=f"xg{g}", tag=f"xg{g}"))
                nc.sync.dma_start(xg[g][:], xt[:, 4 * COLS * g : 4 * COLS * (g + 1)])

            mm = nc.tensor.matmul

            h1 = [None] * NTILES
            h2 = [None] * NTILES
            cs = [None] * NTILES
            sB = [None] * NTILES
            sT = [None] * NTILES
            sqa = [None] * NTILES
            sqb = [None] * NTILES
            h4 = [None] * NTILES
            h5 = [None] * NTILES

            LAG = dict(A=2, B=3, C=4, F=6, G=7, H=8, I=9, J=10, K=11)

            def live(ph, t):
                i = t - LAG[ph]
                return i if 0 <= i < NTILES else None

            for t in range(NTILES + 12):
                # ---- A: h1 = relu(W1 x + b1)   [feature-major]
                i = live("A", t)
                if i is not None:
                    h1p = pmlp.tile([128, COLS], F32, tag="mlp")
                    mm(h1p[:], w1, xg[i // 4][:, COLS * (i % 4) : COLS * (i % 4 + 1)])
                    h1[i] = wp.tile([128, COLS], F16, tag="h1", name="h1")
                    nc.vector.tensor_scalar(h1[i][:], h1p[:], bias["b1"], 0.0, ALU.add, ALU.max)

                # ---- B: h2 = relu(W2 h1 + b2)
                i = live("B", t)
                if i is not None:
                    h2p = pmlp.tile([64, COLS], F32, tag="mlp", padded_shape=[64, COLS])
                    mm(h2p[:], w2, h1[i][:])
                    h2[i] = wp.tile([64, COLS], F16, tag="h2", name="h2")
                    nc.vector.tensor_scalar(h2[i][:], h2p[:], bias["b2"], 0.0, ALU.add, ALU.max)

                # ---- C: feature-major pre-angles, tanh+sin via per-partition
                # bias/scale APs, then XBAR-transpose to batch-major cs
                i = live("C", t)
                if i is not None:
                    prp = pmlp.tile([16, COLS], F32, tag="mlp", padded_shape=[16, COLS])
                    mm(prp[:], w3, h2[i][:])
                    preF = wp.tile([16, COLS], F16, tag="preF", name="preF")
                    nc.scalar.activation(preF[:], prp[:], AF.Tanh, bias=bias["b3"])
                    csF = wp.tile([16, COLS], F16, tag="csF", name="csF")
                    nc.scalar.activation(
                        csF[:], preF[:], AF.Sin, bias=bias["sinb"], scale=bias["sins"]
                    )
                    cs[i] = wp.tile([128, 4, 16], F16, tag="cs", name="cs")
                    nc.sync.dma_start_transpose(cs[i][:], csF[:])

                # ---- F: kron product state build (batch-major)
                i = live("F", t)
                if i is not None:
                    qp = wp.tile([128, 64], F16, tag="qp", name="qp")
                    for a in range(2):
                        nc.gpsimd.tensor_mul(
                            _ap(qp, 2 * a, [[16, 4], [4, 4], [1, 2]]),
                            _ap(cs[i], 8 * a, [[16, 4], [2, 4], [0, 2]]),
                            _ap(cs[i], 1, [[16, 4], [2, 4], [8, 2]]),
                        )
                    uv = wp.tile([128, 128], F16, tag="uv", name="uv")
                    nc.gpsimd.tensor_mul(
                        _ap(uv, 0, [[16, 8], [4, 4], [1, 4]]),
                        _ap(qp, 0, [[8, 8], [1, 4], [0, 4]]),
                        _ap(qp, 4, [[8, 8], [0, 4], [1, 4]]),
                    )
                    # sB column layout is h-major so the XBAR-transposed state
                    # halves come out contiguous: col(b,H,L) =
                    # 512*(H//8) + 128*b + 16*(H%8) + L
                    sB[i] = wp.tile([128, 1024], F16, tag="sB", name="sB", bufs=4)
                    for bp in range(2):
                        for hh in range(2):
                            nc.gpsimd.tensor_mul(
                                _ap(sB[i], 512 * hh + 256 * bp, [[128, 2], [16, 8], [1, 16]]),
                                _ap(uv, 64 * bp + 8 * hh, [[32, 2], [1, 8], [0, 16]]),
                                _ap(uv, 64 * bp + 16, [[32, 2], [0, 8], [1, 16]]),
                            )

                # ---- G: XBAR DMA transpose sB [128,1024] -> sT [128,8,128]
                # (per-128-col-block transpose: sT[j, k, p] = sB[p, 128k + j];
                # source block k = 2b + h covers sub-block b, state-half h)
                i = live("G", t)
                if i is not None:
                    sT[i] = wp.tile([128, 8, 128], F16, tag="sT", name="sT", bufs=4)
                    nc.sync.dma_start_transpose(sT[i][:], sB[i][:])

                # ---- H + squares: y = C s per output chunk, square on ACT
                i = live("H", t)
                if i is not None:
                    sqa[i] = wp.tile([128, 1024], F16, tag="sqa", name="sqa")
                    sqb[i] = wp.tile([128, 1024], F16, tag="sqb", name="sqb")
                    for mc in range(4):
                        yp = py.tile([128, COLS], F32, tag="y")
                        mm(yp[:], ct[:, 128 * mc : 128 * (mc + 1)],
                           _ap(sT[i], 0, [[1, 512]]), start=True, stop=False)
                        mm(yp[:], ct[:, 512 + 128 * mc : 512 + 128 * (mc + 1)],
                           _ap(sT[i], 512, [[1, 512]]), start=False, stop=True)
                        dst = (sqa if mc < 2 else sqb)[i][:, 512 * (mc % 2) : 512 * (mc % 2 + 1)]
                        nc.scalar.activation(dst, yp[:], AF.Square)

                # ---- I: h4 = relu(y^2 @ M4 + b4)
                i = live("I", t)
                if i is not None:
                    h4p = pmlb.tile([32, COLS], F32, tag="mlb", padded_shape=[32, COLS])
                    for mc in range(4):
                        srct = (sqa if mc < 2 else sqb)[i][:, 512 * (mc % 2) : 512 * (mc % 2 + 1)]
                        mm(h4p[:], m4[:, 32 * mc : 32 * (mc + 1)], srct,
                           start=(mc == 0), stop=(mc == 3))
                    h4[i] = wp.tile([32, COLS], F16, tag="h4", name="h4")
                    nc.vector.tensor_scalar(h4[i][:], h4p[:], bias["b4"], 0.0, ALU.add, ALU.max)

                # ---- J: h5 = relu(W5 h4 + b5)
                i = live("J", t)
                if i is not None:
                    h5p = pmlb.tile([16, COLS], F32, tag="mlb", padded_shape=[16, COLS])
                    mm(h5p[:], w5, h4[i][:])
                    h5[i] = wp.tile([16, COLS], F16, tag="h5", name="h5")
                    nc.vector.tensor_scalar(h5[i][:], h5p[:], bias["b5"], 0.0, ALU.add, ALU.max)

                # ---- K: out = W6 h5 + b6
                i = live("K", t)
                if i is not None:
                    op = pmlb.tile([1, COLS], F32, tag="mlb", padded_shape=[1, COLS])
                    mm(op[:], w6, h5[i][:])
                    outs = wp.tile([1, COLS], F32, tag="outs", name="outs", bufs=2)
                    nc.vector.tensor_copy(outs[:], op[:])
                    nc.sync.dma_start(out_d[:, COLS * i : COLS * (i + 1)], outs[:])

    nc.compile()
    return nc


_NC_CACHE = []

# test-harness hooks (unused in grading): set _TRACE to profile; the full
# BassKernelResults of the last run lands in _LAST_RESULTS[0].
_TRACE = False
_LAST_RESULTS = []


def _get_nc():
    if not _NC_CACHE:
        _NC_CACHE.append(_build_nc())
    return _NC_CACHE[0]


def kernel(**inputs):
    consts = _prep_consts(inputs)
    x = np.asarray(inputs["x"], np.float32)  # (65536, 16)
    xt_full = np.ascontiguousarray(x.T.astype(np.float16))  # (16, 65536)

    nc = _get_nc()
    in_maps = []
    for c in range(N_CORES):
        m = {"xt": np.ascontiguousarray(xt_full[:, c * B_CORE : (c + 1) * B_CORE])}
        m.update(consts)
        in_maps.append(m)
    res = run_bass_kernel_spmd(nc, in_maps, list(range(N_CORES)), trace=_TRACE)
    _LAST_RESULTS.clear()
    _LAST_RESULTS.append(res)
    out = np.concatenate([r["out"].reshape(B_CORE) for r in res.results])
    out = out + np.float32(np.asarray(inputs["b6"], np.float32).reshape(()))
    return out.reshape(BATCH, 1).astype(np.float32)


# revision 42
# speedup vs baseline: 1.2435x; 1.2435x over previous
"""Trainium2 kernel for nn_EnhancedHybridModel (hybrid MLP + 8-qubit circuit).

Reformulation (exact, up to f32 rounding):
  * BatchNorms are inference-mode -> folded into the adjacent Linear.
  * The quantum circuit after RY-encoding uses shared weights, so it is one
    fixed complex matrix M (256x256).  The encoded state is a REAL product
    vector s[b] = kron_i [cos(pre_i/2), -sin(pre_i/2)].
  * q_out = |M s|^2 @ Z  ->  y = [Re M; Im M] @ s  (512x256 matmul),
    then q_out @ W4eff.T folds with the Z-projection into M4 (512x32):
    h4 = relu(y^2 @ M4 + b4eff).

Data parallel over 8 NeuronCores: batch 65536 -> 8192 rows/core.
v2 pipeline: the pre-angles are computed BATCH-major (stationary = h2
sub-blocks) so cos/sin come from two scalar-engine Sin LUT ops and no
feature-major->batch-major transposes are needed.  The product-state build
stays in batch-major (strided broadcast krons on DVE/Pool), is transposed
back by 8 PE-transposes into one fp16 PSUM bank, and evacuated by a single
DVE copy.  All matmuls fp16.
"""

import numpy as np

import concourse.bass as bass
import concourse.mybir as mybir
import concourse.tile as tile
from concourse import bacc
from concourse.hw_specs import get_activation_tables
from concourse.masks import make_identity
from concourse.bass_utils import run_bass_kernel_spmd

F32 = mybir.dt.float32
F16 = mybir.dt.float16
AF = mybir.ActivationFunctionType
ALU = mybir.AluOpType

N_CORES = 8
BATCH = 65536
B_CORE = BATCH // N_CORES  # 8192
COLS = 512  # batch columns per tile (one PSUM bank of f32)
NTILES = B_CORE // COLS  # 16

N_QUBITS = 8
N_LAYERS = 3
DIM = 256
EPS = 1e-5

# ---------------------------------------------------------------- host math

_idx = np.arange(DIM)
_CNOT_PERMS = []
for _i in range(N_QUBITS):
    for _j in range(_i + 1, N_QUBITS):
        _c = (_idx >> (N_QUBITS - 1 - _i)) & 1
        _CNOT_PERMS.append(np.where(_c == 1, _idx ^ (1 << (N_QUBITS - 1 - _j)), _idx))
_Z_SIGNS = np.stack(
    [1.0 - 2.0 * ((_idx >> (N_QUBITS - 1 - i)) & 1) for i in range(N_QUBITS)], axis=1
).astype(np.float64)


def _rx(t):
    c, s = np.cos(t / 2), -1j * np.sin(t / 2)
    return np.array([[c, s], [s, c]], np.complex128)


def _ry(t):
    c, s = np.cos(t / 2), np.sin(t / 2)
    return np.array([[c, -s], [s, c]], np.complex128)


def _rz(t):
    e = np.exp(-0.5j * t)
    return np.array([[e, 0], [0, np.conj(e)]], np.complex128)


def _apply_gate(M, G, w):
    # reference einsum('st,bpsq->bptq', U, state): state'[t] = sum_s U[s,t] state[s]
    left = 2**w
    Mr = M.reshape(left, 2, -1, DIM)
    return np.einsum("st,psqj->ptqj", G, Mr).reshape(DIM, DIM)


def _build_circuit_matrix(q_weights):
    qw = np.asarray(q_weights, np.float64)
    M = np.eye(DIM, dtype=np.complex128)
    for l in range(N_LAYERS):
        for i in range(N_QUBITS):
            M = _apply_gate(M, _rx(qw[l, i, 0]), i)
            M = _apply_gate(M, _ry(qw[l, i, 1]), i)
            M = _apply_gate(M, _rz(qw[l, i, 2]), i)
        for perm in _CNOT_PERMS:
            M = M[perm, :]
    return M


def _fold_bn(W, b, g, bt, m, v):
    sc = np.asarray(g, np.float64) / np.sqrt(np.asarray(v, np.float64) + EPS)
    Weff = sc[:, None] * np.asarray(W, np.float64)
    beff = (np.asarray(b, np.float64) - np.asarray(m, np.float64)) * sc + np.asarray(
        bt, np.float64
    )
    return Weff, beff


def _prep_consts(inputs):
    f = {k: np.asarray(v, np.float64) for k, v in inputs.items() if k != "x"}
    W1e, b1e = _fold_bn(f["W1"], f["b1"], f["g1"], f["bt1"], f["m1"], f["v1"])
    W2e, b2e = _fold_bn(f["W2"], f["b2"], f["g2"], f["bt2"], f["m2"], f["v2"])
    W4e, b4e = _fold_bn(f["W4"], f["b4"], f["g4"], f["bt4"], f["m4"], f["v4"])
    M = _build_circuit_matrix(f["q_weights"])
    C = np.concatenate([M.real, M.imag], axis=0)  # (512, 256)
    Zst = np.concatenate([_Z_SIGNS, _Z_SIGNS], axis=0)  # (512, 8)
    M4 = Zst @ W4e.T  # (512, 32)

    bf = np.float16
    # WPACK fp16 [128, 1377]: ct | w2 | w1 | w3 | w5 | w6 | m4  (row-padded)
    wpk = np.zeros((128, 1377), bf)
    CT = np.ascontiguousarray(C.T).astype(bf)  # (256,512)
    wpk[:, 0:512] = CT[0:128]
    wpk[:, 512:1024] = CT[128:256]
    wpk[0:128, 1024:1088] = np.ascontiguousarray(W2e.T).astype(bf)
    wpk[0:16, 1088:1216] = np.ascontiguousarray(W1e.T).astype(bf)
    w3t = np.concatenate([f["W3"].T, f["W3"].T], axis=1)  # (64,16)
    wpk[0:64, 1216:1232] = np.ascontiguousarray(w3t).astype(bf)
    # row 64: b3 (duplicated) — multiplied by the ones-row of the h2 tile so
    # the C' matmul adds the bias for free
    wpk[64, 1216:1232] = np.concatenate([f["b3"], f["b3"]]).astype(bf)
    wpk[0:32, 1232:1248] = np.ascontiguousarray(f["W5"].T).astype(bf)
    wpk[0:16, 1248:1249] = np.ascontiguousarray(f["W6"].T).astype(bf)
    M4b = M4.astype(bf)  # (512,32)
    for c in range(4):
        wpk[:, 1249 + 32 * c : 1249 + 32 * (c + 1)] = M4b[128 * c : 128 * (c + 1)]
    # BIASES f32 [128, 16]: b1 b2 b3 b4 b5 b6
    bs = np.zeros((128, 16), np.float32)
    bs[0:128, 0] = b1e
    bs[0:64, 1] = b2e
    bs[0:16, 2] = np.concatenate([f["b3"], f["b3"]])
    bs[0:32, 3] = b4e
    bs[0:16, 4] = f["b5"]
    bs[0:1, 5] = f["b6"]
    bs[0:128, 6] = np.pi / 2  # cos-as-sin phase offset (batch-major)
    return {"WPACK": wpk, "BIASES": bs}


# ------------------------------------------------------------- bass program


def _ap(t, offset, dims):
    """Custom free-dim access pattern on a tile: keep its partition dim."""
    a = t[:]
    return bass.AP(a.tensor, a.offset + offset, [list(a.ap[0])] + [list(d) for d in dims])


HALF_PI = float(np.pi / 2)


def _build_nc():
    nc = bacc.Bacc("TRN2", target_bir_lowering=False, debug=False)

    xt = nc.dram_tensor("xt", [16, B_CORE], F16, kind="ExternalInput")
    wpk_d = nc.dram_tensor("WPACK", [128, 1377], F16, kind="ExternalInput")
    bs_d = nc.dram_tensor("BIASES", [128, 16], F32, kind="ExternalInput")
    out_d = nc.dram_tensor("out", [1, B_CORE], F32, kind="ExternalOutput")

    with tile.TileContext(nc) as tc:
        with (
            tc.tile_pool(name="const", bufs=1) as cp,
            tc.tile_pool(name="work", bufs=4) as wp,
            tc.tile_pool(name="pmlp", bufs=2, space="PSUM") as pmlp,
            tc.tile_pool(name="ptr", bufs=2, space="PSUM") as ptr,
            tc.tile_pool(name="py", bufs=2, space="PSUM") as py,
            tc.tile_pool(name="pmlb", bufs=2, space="PSUM") as pmlb,
        ):
            # Pre-load one activation table set covering every LUT we use, so
            # the fixpoint table-load pass doesn't thrash between sets
            # (Tanh->exp_and_others vs Sin->trig_and_small at 1283ns/reload).
            _need = {AF.Tanh, AF.Sin, AF.Square, AF.Identity}
            _tabs = list(get_activation_tables(nc.m.arch).items())
            _set_id = next(i for i, (_n, _s) in enumerate(_tabs) if _need <= _s)
            nc.scalar.add_instruction(
                mybir.InstLoadActFuncSet(
                    name=nc.get_next_instruction_name(), ins=[], outs=[],
                    act_func_set_id=_set_id,
                )
            )

            wpk = cp.tile([128, 1377], F16)
            nc.scalar.dma_start(wpk[:], wpk_d[:])
            bs = cp.tile([128, 16], F32)
            nc.sync.dma_start(bs[:], bs_d[:])
            ct = wpk[:, 0:1024]
            w2 = wpk[:, 1024:1088]
            w1 = wpk[0:16, 1088:1216]
            w3 = wpk[0:65, 1216:1232]
            w5 = wpk[0:32, 1232:1248]
            w6 = wpk[0:16, 1248:1249]
            m4 = wpk[:, 1249:1377]
            bias = {
                "b1": bs[0:128, 0:1], "b2": bs[0:64, 1:2],
                "b4": bs[0:32, 3:4], "b5": bs[0:16, 4:5],
                "hpi": bs[0:128, 6:7],
            }
            xg = []
            for g in range(4):
                xg.append(cp.tile([16, 4 * COLS], F16, name=f"xg{g}", tag=f"xg{g}"))
                nc.sync.dma_start(xg[g][:], xt[:, 4 * COLS * g : 4 * COLS * (g + 1)])
            out_all = cp.tile([1, B_CORE], F32)

            mm = nc.tensor.matmul

            h1 = [None] * NTILES
            h2 = [None] * NTILES
            pCs = [None] * NTILES
            cs = [None] * NTILES
            sB = [None] * NTILES
            sT = [None] * NTILES
            sqa = [None] * NTILES
            sqb = [None] * NTILES
            h4 = [None] * NTILES
            h5 = [None] * NTILES

            LAG = dict(A=2, B=3, C=4, P=5, F=6, G=7, H=8, I=9, J=10, K=11)

            def live(ph, t):
                i = t - LAG[ph]
                return i if 0 <= i < NTILES else None

            for t in range(NTILES + 12):
                # ---- A: h1 = relu(W1 x + b1)   [feature-major]
                i = live("A", t)
                if i is not None:
                    h1p = pmlp.tile([128, COLS], F32, tag="mlp")
                    mm(h1p[:], w1, xg[i // 4][:, COLS * (i % 4) : COLS * (i % 4 + 1)])
                    h1[i] = wp.tile([128, COLS], F16, tag="h1", name="h1")
                    nc.vector.tensor_scalar(h1[i][:], h1p[:], bias["b1"], 0.0, ALU.add, ALU.max)

                # ---- B: h2 = relu(W2 h1 + b2); row 64 = ones (bias row for C')
                i = live("B", t)
                if i is not None:
                    h2p = pmlp.tile([64, COLS], F32, tag="mlp", padded_shape=[64, COLS])
                    mm(h2p[:], w2, h1[i][:])
                    h2[i] = wp.tile([65, COLS], F16, tag="h2", name="h2")
                    if i < 4:  # ring has 4 bufs; ones-row persists across reuse
                        nc.vector.memset(h2[i][64:65, :], 1.0)
                    nc.vector.tensor_scalar(h2[i][0:64, :], h2p[:], bias["b2"], 0.0, ALU.add, ALU.max)

                # ---- C: batch-major pre-angles (b3 added via h2's ones-row)
                i = live("C", t)
                if i is not None:
                    pCs[i] = ptr.tile([128, 64], F32, tag="tr", name="pC",
                                      padded_shape=[128, COLS])
                    for b in range(4):
                        mm(pCs[i][:, 16 * b : 16 * (b + 1)],
                           h2[i][0:65, 128 * b : 128 * (b + 1)], w3)

                # ---- P: tanh then sin/cos straight off the PSUM
                i = live("P", t)
                if i is not None:
                    preT = wp.tile([128, 64], F16, tag="preT", name="preT")
                    nc.scalar.activation(preT[:], pCs[i][:], AF.Tanh)
                    cs[i] = wp.tile([128, 64], F16, tag="cs", name="cs")
                    nc.scalar.activation(
                        _ap(cs[i], 0, [[16, 4], [1, 8]]),
                        _ap(preT, 0, [[16, 4], [1, 8]]),
                        AF.Sin, bias=bias["hpi"], scale=0.5,
                    )
                    nc.scalar.activation(
                        _ap(cs[i], 8, [[16, 4], [1, 8]]),
                        _ap(preT, 8, [[16, 4], [1, 8]]),
                        AF.Sin, bias=0.0, scale=-0.5,
                    )

                # ---- F: kron product state build (batch-major)
                i = live("F", t)
                if i is not None:
                    qp = wp.tile([128, 64], F16, tag="qp", name="qp")
                    for a in range(2):
                        nc.gpsimd.tensor_mul(
                            _ap(qp, 2 * a, [[16, 4], [4, 4], [1, 2]]),
                            _ap(cs[i], 8 * a, [[16, 4], [2, 4], [0, 2]]),
                            _ap(cs[i], 1, [[16, 4], [2, 4], [8, 2]]),
                        )
                    uv = wp.tile([128, 128], F16, tag="uv", name="uv")
                    nc.gpsimd.tensor_mul(
                        _ap(uv, 0, [[16, 8], [4, 4], [1, 4]]),
                        _ap(qp, 0, [[8, 8], [1, 4], [0, 4]]),
                        _ap(qp, 4, [[8, 8], [0, 4], [1, 4]]),
                    )
                    # sB column layout is h-major so the XBAR-transposed state
                    # halves come out contiguous: col(b,H,L) =
                    # 512*(H//8) + 128*b + 16*(H%8) + L
                    sB[i] = wp.tile([128, 1024], F16, tag="sB", name="sB", bufs=4)
                    for bp in range(2):
                        for hh in range(2):
                            nc.gpsimd.tensor_mul(
                                _ap(sB[i], 512 * hh + 256 * bp, [[128, 2], [16, 8], [1, 16]]),
                                _ap(uv, 64 * bp + 8 * hh, [[32, 2], [1, 8], [0, 16]]),
                                _ap(uv, 64 * bp + 16, [[32, 2], [0, 8], [1, 16]]),
                            )

                # ---- G: XBAR DMA transpose sB [128,1024] -> sT [128,8,128]
                # (per-128-col-block transpose: sT[j, k, p] = sB[p, 128k + j];
                # source block k = 2b + h covers sub-block b, state-half h)
                i = live("G", t)
                if i is not None:
                    sT[i] = wp.tile([128, 8, 128], F16, tag="sT", name="sT", bufs=4)
                    nc.sync.dma_start_transpose(sT[i][:], sB[i][:])

                # ---- H + squares: y = C s per output chunk, square on ACT
                i = live("H", t)
                if i is not None:
                    sqa[i] = wp.tile([128, 1024], F16, tag="sqa", name="sqa")
                    sqb[i] = wp.tile([128, 1024], F16, tag="sqb", name="sqb")
                    for mc in range(4):
                        yp = py.tile([128, COLS], F32, tag="y")
                        mm(yp[:], ct[:, 128 * mc : 128 * (mc + 1)],
                           _ap(sT[i], 0, [[1, 512]]), start=True, stop=False)
                        mm(yp[:], ct[:, 512 + 128 * mc : 512 + 128 * (mc + 1)],
                           _ap(sT[i], 512, [[1, 512]]), start=False, stop=True)
                        dst = (sqa if mc < 2 else sqb)[i][:, 512 * (mc % 2) : 512 * (mc % 2 + 1)]
                        nc.scalar.activation(dst, yp[:], AF.Square)

                # ---- I: h4 = relu(y^2 @ M4 + b4)
                i = live("I", t)
                if i is not None:
                    h4p = pmlb.tile([32, COLS], F32, tag="mlb", padded_shape=[32, COLS])
                    for mc in range(4):
                        srct = (sqa if mc < 2 else sqb)[i][:, 512 * (mc % 2) : 512 * (mc % 2 + 1)]
                        mm(h4p[:], m4[:, 32 * mc : 32 * (mc + 1)], srct,
                           start=(mc == 0), stop=(mc == 3))
                    h4[i] = wp.tile([32, COLS], F16, tag="h4", name="h4")
                    nc.vector.tensor_scalar(h4[i][:], h4p[:], bias["b4"], 0.0, ALU.add, ALU.max)

                # ---- J: h5 = relu(W5 h4 + b5)
                i = live("J", t)
                if i is not None:
                    h5p = pmlb.tile([16, COLS], F32, tag="mlb", padded_shape=[16, COLS])
                    mm(h5p[:], w5, h4[i][:])
                    h5[i] = wp.tile([16, COLS], F16, tag="h5", name="h5")
                    nc.vector.tensor_scalar(h5[i][:], h5p[:], bias["b5"], 0.0, ALU.add, ALU.max)

                # ---- K: out = W6 h5 + b6
                i = live("K", t)
                if i is not None:
                    op = pmlb.tile([1, COLS], F32, tag="mlb", padded_shape=[1, COLS])
                    mm(op[:], w6, h5[i][:])
                    nc.vector.tensor_copy(out_all[0:1, COLS * i : COLS * (i + 1)], op[:])
                    if i % 4 == 3:  # quarter-batch output DMAs overlap the drain
                        nc.sync.dma_start(
                            out_d[:, COLS * (i - 3) : COLS * (i + 1)],
                            out_all[0:1, COLS * (i - 3) : COLS * (i + 1)],
                        )

    nc.compile()
    return nc


_NC_CACHE = []

# test-harness hooks (unused in grading): set _TRACE to profile; the full
# BassKernelResults of the last run lands in _LAST_RESULTS[0].
_TRACE = False
_LAST_RESULTS = []


def _get_nc():
    if not _NC_CACHE:
        _NC_CACHE.append(_build_nc())
    return _NC_CACHE[0]


def kernel(**inputs):
    consts = _prep_consts(inputs)
    x = np.asarray(inputs["x"], np.float32)  # (65536, 16)
    xt_full = np.ascontiguousarray(x.T.astype(np.float16))  # (16, 65536)

    nc = _get_nc()
    in_maps = []
    for c in range(N_CORES):
        m = {"xt": np.ascontiguousarray(xt_full[:, c * B_CORE : (c + 1) * B_CORE])}
        m.update(consts)
        in_maps.append(m)
    res = run_bass_kernel_spmd(nc, in_maps, list(range(N_CORES)), trace=_TRACE)
    _LAST_RESULTS.clear()
    _LAST_RESULTS.append(res)
    out = np.concatenate([r["out"].reshape(B_CORE) for r in res.results])
    out = out + np.float32(np.asarray(inputs["b6"], np.float32).reshape(()))
    return out.reshape(BATCH, 1).astype(np.float32)
